# revision 14
# baseline (speedup 1.0000x reference)
"""Trainium2 Bass kernel: 3x depthwise-separable conv blocks + BN(batch stats) + ReLU + global avgpool.

Sharding: data-parallel over batch (32 imgs -> 4 per core x 8 cores).
BN batch statistics are made exact via on-device AllReduce of per-channel
(sum, sum_sq) across the 8 cores.

Compute layout per core (4 local images n=0..3):
  block0: partitions p=(n*32+c)      [128], spatial 112x112 (padded 114x116 in SBUF)
  block1: partitions p=(n_loc*64+c)  [128], 2 image groups {0,1},{2,3}, spatial 112->56
  block2: partitions p=c             [128], 4 image groups, spatial 56
Depthwise 3x3 conv = 9 diagonal-matmul taps accumulated in PSUM (bf16).
Pointwise 1x1 conv = dense matmul with host-prebuilt block-diagonal weights (bf16).
"""

import numpy as np
import ml_dtypes

import concourse.bass as bass
import concourse.bacc as bacc
import concourse.tile as tile
from concourse import mybir
from concourse.bass_utils import run_bass_kernel_spmd

F32 = mybir.dt.float32
BF16 = mybir.dt.bfloat16
AF = mybir.ActivationFunctionType
ALU = mybir.AluOpType

N_CORES = 8
EPS = 1e-5

TRACE = False          # set by test.py to capture HW profile
LAST_RESULTS = None    # BassKernelResults of the last run

_PROG = None           # cached compiled program


# ----------------------------------------------------------------------------- host-side weight prep

def _bf16(a):
    return np.ascontiguousarray(np.asarray(a, np.float32)).astype(ml_dtypes.bfloat16)


def _build_host_weights(inputs):
    w = {}
    # --- depthwise diagonal weight mats: [9, 128, 128] per block
    for b, rep in ((0, 32), (1, 64), (2, 128)):
        dw = np.asarray(inputs[f"b{b}_dw_w"], np.float32)[:, 0]  # [cin,3,3]
        mats = np.zeros((9, 128, 128), np.float32)
        for t in range(9):
            dy, dx = t // 3, t % 3
            diag = dw[np.arange(128) % rep, dy, dx]
            mats[t, np.arange(128), np.arange(128)] = diag
        w[f"dwd{b}"] = _bf16(mats)

    # --- pointwise lhsT mats
    pw0 = np.asarray(inputs["b0_pw_w"], np.float32)  # [64, 32]
    m0 = np.zeros((2, 128, 128), np.float32)
    for g in range(2):
        for k in range(128):
            n, c = k // 32, k % 32
            for m in range(128):
                nl, o = m // 64, m % 64
                if n == 2 * g + nl:
                    m0[g, k, m] = pw0[o, c]
    w["pwm0"] = _bf16(m0)

    pw1 = np.asarray(inputs["b1_pw_w"], np.float32)  # [128, 64]
    m1 = np.zeros((2, 128, 128), np.float32)
    for h in range(2):
        for k in range(128):
            nl, c = k // 64, k % 64
            if nl == h:
                m1[h, k, :] = pw1[:, c]
    w["pwm1"] = _bf16(m1)

    pw2 = np.asarray(inputs["b2_pw_w"], np.float32)  # [128, 128]
    w["pwm2"] = _bf16(pw2.T[None])

    # --- per-partition vectors [18, 128] fp32
    vecs = np.zeros((18, 128), np.float32)
    p = np.arange(128)
    vecs[0] = np.asarray(inputs["b0_dw_b"])[p % 32]
    vecs[1] = np.asarray(inputs["b0_g1"])[p % 32]
    vecs[2] = np.asarray(inputs["b0_be1"])[p % 32]
    vecs[3] = np.asarray(inputs["b0_pw_b"])[p % 64]
    vecs[4] = np.asarray(inputs["b0_g2"])[p % 64]
    vecs[5] = np.asarray(inputs["b0_be2"])[p % 64]
    vecs[6] = np.asarray(inputs["b1_dw_b"])[p % 64]
    vecs[7] = np.asarray(inputs["b1_g1"])[p % 64]
    vecs[8] = np.asarray(inputs["b1_be1"])[p % 64]
    vecs[9] = np.asarray(inputs["b1_pw_b"])[p]
    vecs[10] = np.asarray(inputs["b1_g2"])[p]
    vecs[11] = np.asarray(inputs["b1_be2"])[p]
    vecs[12] = np.asarray(inputs["b2_dw_b"])[p]
    vecs[13] = np.asarray(inputs["b2_g1"])[p]
    vecs[14] = np.asarray(inputs["b2_be1"])[p]
    vecs[15] = np.asarray(inputs["b2_pw_b"])[p]
    vecs[16] = np.asarray(inputs["b2_g2"])[p]
    vecs[17] = np.asarray(inputs["b2_be2"])[p]
    w["vecs"] = vecs
    return w


# ----------------------------------------------------------------------------- bass program

def _chunk_triples(total, clen):
    """Split [0,total) into chunks of clen (last ragged), grouped in runs of <=3 equal-length chunks."""
    chunks = []
    off = 0
    while off < total:
        l = min(clen, total - off)
        chunks.append((off, l))
        off += l
    groups = []
    i = 0
    while i < len(chunks):
        g = [chunks[i]]
        while len(g) < 3 and i + len(g) < len(chunks) and chunks[i + len(g)][1] == g[0][1]:
            g.append(chunks[i + len(g)])
        groups.append(g)
        i += len(g)
    return groups


def _build_program():
    nc = bacc.Bacc(None, target_bir_lowering=False, num_devices=N_CORES)

    x_in = nc.dram_tensor("x", [128, 112, 112], F32, kind="ExternalInput")
    dwd = [nc.dram_tensor(f"dwd{b}", [9, 128, 128], BF16, kind="ExternalInput") for b in range(3)]
    pwm = [nc.dram_tensor(f"pwm{b}", [pwn, 128, 128], BF16, kind="ExternalInput")
           for b, pwn in ((0, 2), (1, 2), (2, 1))]
    vecs_t = nc.dram_tensor("vecs", [18, 128], F32, kind="ExternalInput")
    out_t = nc.dram_tensor("out", [4, 128], F32, kind="ExternalOutput")

    cc_in = [nc.dram_tensor(f"ccin{i}", [128, 2], F32, kind="Internal") for i in range(6)]
    cc_out = [nc.dram_tensor(f"ccout{i}", [128, 2], F32, kind="Internal", addr_space="Shared")
              for i in range(6)]
    RG = [list(range(N_CORES))]

    with tile.TileContext(nc) as tc:
        from contextlib import ExitStack
        with ExitStack() as ctx:
            singles = ctx.enter_context(tc.tile_pool(name="singles", bufs=1))
            small = ctx.enter_context(tc.tile_pool(name="small", bufs=7))
            stats_p = ctx.enter_context(tc.tile_pool(name="stats", bufs=2))
            psum_p = ctx.enter_context(tc.tile_pool(name="psum", bufs=2, space="PSUM"))
            stage_p = ctx.enter_context(tc.tile_pool(name="stage", bufs=7))
            junk_p = ctx.enter_context(tc.tile_pool(name="junk", bufs=2))

            # ---- load constants
            dwW = []
            for b in range(3):
                t_ = singles.tile([128, 9, 128], BF16, tag=f"dwW{b}")
                nc.gpsimd.dma_start(out=t_[:], in_=dwd[b][:].rearrange("t k m -> k t m"))
                dwW.append(t_)
            pwW = []
            for b, pwn in ((0, 2), (1, 2), (2, 1)):
                t_ = singles.tile([128, pwn, 128], BF16, tag=f"pwW{b}")
                nc.gpsimd.dma_start(out=t_[:], in_=pwm[b][:].rearrange("n k m -> k n m"))
                pwW.append(t_)
            vec = singles.tile([128, 18], F32, tag="vec")
            nc.gpsimd.dma_start(out=vec[:], in_=vecs_t[:].rearrange("v p -> p v"))

            def vap(i):
                return vec[:, i:i + 1]

            epsv = singles.tile([128, 1], F32, tag="epsv")
            nc.vector.memset(epsv[:], EPS)

            # ---- helpers ------------------------------------------------------

            def memset_pad(buf, grp, H, W):
                # buf [128, grp, H+2, W+4]; zero rows 0 and H+1, and the 4 pad cols
                nc.vector.memset(buf[:, :, 0:H + 2:H + 1, :], 0.0)
                nc.vector.memset(buf[:, :, :, 0:2], 0.0)
                nc.vector.memset(buf[:, :, :, W + 2:W + 4], 0.0)

            def emit_dw(src_pad, n_grp, Ho, stride, dwW_b, bias_ap, dst, st6, chunk_rows, st_idx0=0):
                Wo = Ho
                cpc = chunk_rows * Wo
                nchunks = Ho // chunk_rows
                sidx = st_idx0
                for g in range(n_grp):
                    ci = 0
                    while ci < nchunks:
                        tri = list(range(ci, min(ci + 3, nchunks)))
                        ps = psum_p.tile([128, 3, 512], F32, tag="ps")
                        for t in range(9):
                            dy, dx = t // 3, t % 3
                            for j, cj in enumerate(tri):
                                r0 = cj * chunk_rows
                                if stride == 1:
                                    rhs = src_pad[:, g, r0 + dy: r0 + dy + chunk_rows,
                                                  dx + 1: dx + 1 + Wo]
                                else:
                                    rhs = src_pad[:, g,
                                                  2 * r0 + dy: 2 * r0 + dy + 2 * chunk_rows: 2,
                                                  dx + 1: dx + 1 + 2 * Wo: 2]
                                nc.tensor.matmul(ps[:, j, 0:cpc], dwW_b[:, t, :], rhs,
                                                 start=(t == 0), stop=(t == 8))
                        region = dst[:, g, tri[0] * cpc:(tri[-1] + 1) * cpc]
                        nc.scalar.activation(
                            out=region.rearrange("p (t c) -> p t c", c=cpc),
                            in_=ps[:, 0:len(tri), 0:cpc],
                            func=AF.Identity, bias=bias_ap, scale=1.0)
                        for cj in tri:
                            nc.vector.bn_stats(out=st6[:, sidx, :],
                                               in_=dst[:, g, cj * cpc:(cj + 1) * cpc])
                            sidx += 1
                        ci += len(tri)
                return sidx - st_idx0

            def emit_pw(srcn, mats, pwW_b, bias_ap, dst, st6, free_len, chunk_cols, st_idx0=0):
                # mats: list of (g_src, mat_idx, g_dst)
                sidx = st_idx0
                for gs, mi, gd in mats:
                    for tri in _chunk_triples(free_len, chunk_cols):
                        ps = psum_p.tile([128, 3, 512], F32, tag="ps")
                        for j, (off, ln) in enumerate(tri):
                            nc.tensor.matmul(ps[:, j, 0:ln], pwW_b[:, mi, :],
                                             srcn[:, gs, off:off + ln],
                                             start=True, stop=True)
                        ln = tri[0][1]
                        region = dst[:, gd, tri[0][0]: tri[-1][0] + tri[-1][1]]
                        nc.scalar.activation(
                            out=region.rearrange("p (t c) -> p t c", c=ln),
                            in_=ps[:, 0:len(tri), 0:ln],
                            func=AF.Identity, bias=bias_ap, scale=1.0)
                        for off, l2 in tri:
                            nc.vector.bn_stats(out=st6[:, sidx, :], in_=dst[:, gd, off:off + l2])
                            sidx += 1
                return sidx - st_idx0

            def emit_bn_params(st6, nch, npart, ntot, cci, gather_ap, nred, gamma, beta):
                mv = small.tile([128, 2], F32, tag="mv")
                nc.vector.bn_aggr(out=mv[:], in_=st6[:, 0:nch, :])
                s = small.tile([128, 2], F32, tag="ssum")
                nc.vector.tensor_scalar(out=s[:, 0:1], in0=mv[:, 0:1],
                                        scalar1=float(npart), scalar2=None, op0=ALU.mult)
                msq = small.tile([128, 1], F32, tag="msq")
                nc.vector.tensor_mul(msq[:], mv[:, 0:1], mv[:, 0:1])
                v1 = small.tile([128, 1], F32, tag="v1")
                nc.vector.tensor_add(v1[:], mv[:, 1:2], msq[:])
                nc.vector.tensor_scalar(out=s[:, 1:2], in0=v1[:],
                                        scalar1=float(npart), scalar2=None, op0=ALU.mult)
                nc.gpsimd.dma_start(out=cc_in[cci][:], in_=s[:])
                nc.gpsimd.collective_compute(
                    "AllReduce", ALU.add, replica_groups=RG,
                    ins=[cc_in[cci][:]], outs=[cc_out[cci][:]])
                allst = small.tile([128, nred, 2], F32, tag="allst")
                part_dims, off_stride = gather_ap
                for n_ in range(nred):
                    src = bass.AP(tensor=cc_out[cci], offset=n_ * off_stride,
                                  ap=list(part_dims) + [[1, 2]])
                    nc.gpsimd.dma_start(out=allst[:, n_, :], in_=src)
                tot = small.tile([128, 2], F32, tag="tot")
                if nred > 1:
                    nc.vector.tensor_reduce(
                        out=tot[:], in_=allst[:].rearrange("p n j -> p j n"),
                        axis=mybir.AxisListType.X, op=ALU.add)
                else:
                    nc.vector.tensor_copy(out=tot[:], in_=allst[:, 0, :])
                meang = small.tile([128, 1], F32, tag="meang")
                nc.vector.tensor_scalar(out=meang[:], in0=tot[:, 0:1],
                                        scalar1=1.0 / ntot, scalar2=None, op0=ALU.mult)
                ex2 = small.tile([128, 1], F32, tag="ex2")
                nc.vector.tensor_scalar(out=ex2[:], in0=tot[:, 1:2],
                                        scalar1=1.0 / ntot, scalar2=None, op0=ALU.mult)
                msq2 = small.tile([128, 1], F32, tag="msq2")
                nc.vector.tensor_mul(msq2[:], meang[:], meang[:])
                varg = small.tile([128, 1], F32, tag="varg")
                nc.vector.tensor_sub(varg[:], ex2[:], msq2[:])
                sd = small.tile([128, 1], F32, tag="sd")
                nc.scalar.activation(out=sd[:], in_=varg[:], func=AF.Sqrt, bias=epsv[:], scale=1.0)
                rstd = small.tile([128, 1], F32, tag="rstd")
                nc.vector.reciprocal(out=rstd[:], in_=sd[:])
                scale = small.tile([128, 1], F32, tag="scalev")
                nc.vector.tensor_mul(scale[:], rstd[:], gamma)
                t1 = small.tile([128, 1], F32, tag="t1")
                nc.vector.tensor_mul(t1[:], meang[:], scale[:])
                nbias = small.tile([128, 1], F32, tag="nbias")
                nc.vector.tensor_sub(nbias[:], beta, t1[:])
                return scale, nbias

            def bnapply(engine, dst_ap, src_ap, scale, nbias, accum=None):
                if engine == "act":
                    nc.scalar.activation(out=dst_ap, in_=src_ap, func=AF.Relu,
                                         bias=nbias[:], scale=scale[:], accum_out=accum)
                else:
                    nc.vector.tensor_scalar(out=dst_ap, in0=src_ap, scalar1=scale[:],
                                            scalar2=nbias[:], op0=ALU.mult, op1=ALU.add)
                    nc.vector.tensor_scalar(out=dst_ap, in0=dst_ap, scalar1=0.0,
                                            scalar2=None, op0=ALU.max)

            # gather specs: (partition dims of src AP, offset stride per replica-slice)
            GATH_B0_BN1 = ([[0, 4], [2, 32]], 64)    # (n4,c32) -> sum over 4 n
            GATH_NL2 = ([[0, 2], [2, 64]], 128)      # (nl2,ch64) -> sum over 2 nl
            GATH_DIRECT = ([[2, 128]], 0)            # ch128 -> direct

            # ---- activations: one pool, one tag, bufs=2 -> chain alternates slots
            acts = ctx.enter_context(tc.tile_pool(name="acts", bufs=2))

            # ---- block 0 ------------------------------------------------------
            xpad = acts.tile([128, 1, 114, 116], BF16, tag="act")
            memset_pad(xpad, 1, 112, 112)
            for r in range(7):
                r0 = r * 16
                stg = stage_p.tile([128, 16, 112], F32, tag="xstage")
                nc.gpsimd.dma_start(out=stg[:], in_=x_in[:, r0:r0 + 16, :])
                nc.vector.tensor_copy(out=xpad[:, 0, 1 + r0:1 + r0 + 16, 2:114], in_=stg[:])

            y1b0 = acts.tile([128, 1, 12544], BF16, tag="act")
            st6_0 = stats_p.tile([128, 28, 6], F32, tag="st6")
            emit_dw(xpad, 1, 112, 1, dwW[0], vap(0), y1b0, st6_0, 4)

            sc, nb = emit_bn_params(st6_0, 28, 12544, 401408, 0, GATH_B0_BN1, 4, vap(1), vap(2))

            y1nb0 = acts.tile([128, 1, 12544], BF16, tag="act")
            for k in range(4):
                bnapply("dve", y1nb0[:, 0, k * 3136:(k + 1) * 3136],
                        y1b0[:, 0, k * 3136:(k + 1) * 3136], sc, nb)

            y2b0 = acts.tile([128, 2, 12544], BF16, tag="act")
            st6_1 = stats_p.tile([128, 50, 6], F32, tag="st6")
            emit_pw(y1nb0, [(0, 0, 0), (0, 1, 1)], pwW[0], vap(3), y2b0, st6_1, 12544, 512)

            sc, nb = emit_bn_params(st6_1, 50, 25088, 401408, 1, GATH_NL2, 2, vap(4), vap(5))

            y2nb0 = acts.tile([128, 2, 114, 116], BF16, tag="act")
            memset_pad(y2nb0, 2, 112, 112)
            for g in range(2):
                for k in range(4):
                    r0 = k * 28
                    bnapply("act",
                            y2nb0[:, g, 1 + r0:1 + r0 + 28, 2:114],
                            y2b0[:, g, r0 * 112:(r0 + 28) * 112].rearrange(
                                "p (h w) -> p h w", w=112),
                            sc, nb)

            # ---- block 1 ------------------------------------------------------
            y1b1 = acts.tile([128, 2, 3136], BF16, tag="act")
            st6_2 = stats_p.tile([128, 14, 6], F32, tag="st6")
            emit_dw(y2nb0, 2, 56, 2, dwW[1], vap(6), y1b1, st6_2, 8)

            sc, nb = emit_bn_params(st6_2, 14, 6272, 100352, 2, GATH_NL2, 2, vap(7), vap(8))

            y1nb1 = acts.tile([128, 2, 3136], BF16, tag="act")
            for g in range(2):
                bnapply("dve", y1nb1[:, g, :], y1b1[:, g, :], sc, nb)

            y2b1 = acts.tile([128, 4, 3136], BF16, tag="act")
            st6_3 = stats_p.tile([128, 28, 6], F32, tag="st6")
            emit_pw(y1nb1, [(g, h, 2 * g + h) for g in range(2) for h in range(2)],
                    pwW[1], vap(9), y2b1, st6_3, 3136, 448)

            sc, nb = emit_bn_params(st6_3, 28, 12544, 100352, 3, GATH_DIRECT, 1, vap(10), vap(11))

            y2nb1 = acts.tile([128, 4, 58, 60], BF16, tag="act")
            memset_pad(y2nb1, 4, 56, 56)
            for i in range(4):
                bnapply("dve",
                        y2nb1[:, i, 1:57, 2:58],
                        y2b1[:, i, :].rearrange("p (h w) -> p h w", w=56),
                        sc, nb)

            # ---- block 2 ------------------------------------------------------
            y1b2 = acts.tile([128, 4, 3136], BF16, tag="act")
            st6_4 = stats_p.tile([128, 28, 6], F32, tag="st6")
            emit_dw(y2nb1, 4, 56, 1, dwW[2], vap(12), y1b2, st6_4, 8)

            sc, nb = emit_bn_params(st6_4, 28, 12544, 100352, 4, GATH_DIRECT, 1, vap(13), vap(14))

            y1nb2 = acts.tile([128, 4, 3136], BF16, tag="act")
            for i in range(4):
                bnapply("act", y1nb2[:, i, :], y1b2[:, i, :], sc, nb)

            y2b2 = acts.tile([128, 4, 3136], BF16, tag="act")
            st6_5 = stats_p.tile([128, 28, 6], F32, tag="st6")
            emit_pw(y1nb2, [(i, 0, i) for i in range(4)], pwW[2], vap(15), y2b2, st6_5, 3136, 448)

            sc, nb = emit_bn_params(st6_5, 28, 12544, 100352, 5, GATH_DIRECT, 1, vap(16), vap(17))

            # final: relu(bn(y2b2)) -> global average pool -> out [4, 128]
            acc = singles.tile([128, 4], F32, tag="acc")
            for i in range(4):
                jk = junk_p.tile([128, 3136], BF16, tag="junk")
                bnapply("act", jk[:], y2b2[:, i, :], sc, nb, accum=acc[:, i:i + 1])
            acc2 = singles.tile([128, 4], F32, tag="acc2")
            nc.vector.tensor_scalar(out=acc2[:], in0=acc[:], scalar1=1.0 / 3136.0,
                                    scalar2=None, op0=ALU.mult)
            nc.gpsimd.dma_start(out=out_t[:].transpose([1, 0]), in_=acc2[:])

    nc.compile()
    return nc


def _get_program():
    global _PROG
    if _PROG is None:
        _PROG = _build_program()
    return _PROG


# ----------------------------------------------------------------------------- entry point

def kernel(**inputs):
    global LAST_RESULTS
    x = np.asarray(inputs["x"], np.float32)  # [32, 32, 112, 112]
    w = _build_host_weights(inputs)
    nc = _get_program()

    in_maps = []
    for core in range(N_CORES):
        xs = np.ascontiguousarray(x[core * 4:(core + 1) * 4].reshape(128, 112, 112))
        m = {"x": xs}
        m.update(w)
        in_maps.append(m)

    res = run_bass_kernel_spmd(nc, in_maps, core_ids=list(range(N_CORES)), trace=TRACE)
    LAST_RESULTS = res
    outs = [r["out"] for r in res.results]
    full = np.concatenate(outs, axis=0).reshape(32, 128, 1, 1).astype(np.float32)
    return full
